# revision 1
# baseline (speedup 1.0000x reference)
"""Trainium2 Bass kernel for CustomGRUModel.

Reference computation (per batch row):
    gx = x @ W                       # [T, 3H] input projections (precomputed)
    per step t:
        gh_zr = h @ U[:, :2H]
        z = sigmoid(gxz + ghz + bz)
        r = sigmoid(gxr + ghr + br)
        n = tanh(gxn + (r*h) @ U[:, 2H:] + bn)
        h = z*h + (1-z)*n
    y = h_last @ Wd + bd

Sharding: data-parallel over batch, 32 rows per core on 8 cores. Weights
replicated. No collectives.

Per-core layout: everything transposed ("feature on partitions"):
  hT [H=512, B=32] stored as one SBUF tile [128, 4*32] (4 H-chunks packed in
  the free dim). Recurrent matmuls keep U as the stationary operand
  (lhsT = U k-tile slice [128, 128], fp32 exact) streaming hT chunks (N=32):
  output lands transposed [3H-chunk, B] in PSUM, which makes the gate
  elementwise work run on full 128 partitions.

The gx precompute runs chunked (16 steps at a time) in float32r (1 cyc/row at
N=512), interleaved between recurrence steps so it fills TensorE gaps. x is
transposed on-chip with PE transposes. The bias b is folded into the
PSUM->SBUF eviction of gx (ACT activation bias).
"""

import os

import numpy as np

B, T, D, H = 256, 512, 256, 512
NCORES = 8
BL = B // NCORES  # 32 batch rows per core
TC = 16  # timestep chunk for the gx precompute
KH = H // 128  # 4 k-tiles over H
KD = D // 128  # 2 k-tiles over D
M3H = 3 * H // 128  # 12 m-tiles over 3H

_CACHE = {}


def _build(t_run):
    from contextlib import ExitStack

    import concourse.bacc as bacc
    import concourse.bass as bass
    import concourse.tile as tile
    from concourse import masks, mybir

    dt = mybir.dt
    f32 = dt.float32
    f32r = dt.float32r
    AF = mybir.ActivationFunctionType

    nchunk = t_run // TC

    nc = bacc.Bacc(
        "TRN2", target_bir_lowering=False, debug=False, num_devices=NCORES
    )
    x_d = nc.dram_tensor("x", [BL, T, D], f32, kind="ExternalInput")
    w_d = nc.dram_tensor("W", [D, 3 * H], f32, kind="ExternalInput")
    u_d = nc.dram_tensor("U", [H, 3 * H], f32, kind="ExternalInput")
    b_d = nc.dram_tensor("b", [3 * H], f32, kind="ExternalInput")
    wd_d = nc.dram_tensor("Wd", [H, 1], f32, kind="ExternalInput")
    bd_d = nc.dram_tensor("bd", [1], f32, kind="ExternalInput")
    y_d = nc.dram_tensor("y", [BL, 1], f32, kind="ExternalOutput")

    # chunked view of x: [chunk, tc, b, d]
    x_view = x_d.rearrange("b (c t) d -> c t b d", t=TC)

    with tile.TileContext(nc) as tc, ExitStack() as ctx:
        const = ctx.enter_context(tc.tile_pool(name="const", bufs=1))
        gx_pool = ctx.enter_context(tc.tile_pool(name="gx", bufs=2))
        xin_pool = ctx.enter_context(tc.tile_pool(name="xin", bufs=8))
        xt_pool = ctx.enter_context(tc.tile_pool(name="xt", bufs=2))
        sb_pool = ctx.enter_context(tc.tile_pool(name="sb", bufs=3))
        zr_psum = ctx.enter_context(
            tc.tile_pool(name="zrp", bufs=2, space=bass.MemorySpace.PSUM)
        )
        n_psum = ctx.enter_context(
            tc.tile_pool(name="np", bufs=2, space=bass.MemorySpace.PSUM)
        )
        pre_psum = ctx.enter_context(
            tc.tile_pool(name="prep", bufs=2, space=bass.MemorySpace.PSUM)
        )
        xt_psum = ctx.enter_context(
            tc.tile_pool(name="xtp", bufs=2, space=bass.MemorySpace.PSUM)
        )

        # ---- constants ----
        w_stage = const.tile([128, KD, 3 * H], f32)
        for k in range(KD):
            nc.sync.dma_start(w_stage[:, k, :], w_d[k * 128 : (k + 1) * 128, :])
        w_sb = const.tile([128, KD, 3 * H], f32r)
        for k in range(KD):
            nc.scalar.copy(w_sb[:, k, :], w_stage[:, k, :])
        u_sb = const.tile([128, KH, 3 * H], f32)
        for k in range(KH):
            nc.sync.dma_start(u_sb[:, k, :], u_d[k * 128 : (k + 1) * 128, :])
        b_sb = const.tile([128, M3H], f32)
        nc.sync.dma_start(b_sb[:], b_d.rearrange("(m p) -> p m", p=128))
        wd_sb = const.tile([128, KH], f32)
        nc.sync.dma_start(wd_sb[:], wd_d.rearrange("(k p) o -> p (k o)", p=128))
        bd_sb = const.tile([1, 1], f32)
        nc.sync.dma_start(bd_sb[0:1, :], bd_d.rearrange("(o u) -> o u", u=1))
        ident = const.tile([128, 128], f32)
        masks.make_identity(nc, ident[:])
        ones_sb = const.tile([1, BL], f32)
        nc.gpsimd.memset(ones_sb[0:1, :], 1.0)

        # persistent hidden state hT: [128, (k, b)] = [128, 4*32]
        h_sb = const.tile([128, KH * BL], f32)
        nc.gpsimd.memset(h_sb[:], 0.0)

        warm_ps = n_psum.tile([128, 128], f32, name="warm", tag="np")
        nc.tensor.transpose(warm_ps[:], ident[:], ident[:])

        gx_tiles = {}

        def make_units(c):
            """Emit-thunks for precomputing gx chunk c (16 steps)."""
            gx_t = gx_pool.tile([128, TC, M3H, BL], f32, name="gx", tag="gx")
            gx_tiles[c] = gx_t
            xins = []
            xt_sb = xt_pool.tile([128, KD, TC * BL], f32r, name="xt", tag="xt")
            xt_ps = {}
            units = []

            def load(j):
                t = xin_pool.tile([128, D], f32, name="xin", tag="xin")
                xins.append(t)
                nc.sync.dma_start(
                    t[:],
                    x_view[c, 4 * j : 4 * (j + 1)],
                )

            def tr(j):
                # transpose both d-chunks of xin row-block j
                for kd in range(KD):
                    if j == 0:
                        xt_ps[kd] = xt_psum.tile([128, TC * BL], f32, name="xtp", tag="xtp")
                    nc.tensor.transpose(
                        xt_ps[kd][:, 128 * j : 128 * (j + 1)],
                        xins[j][:, 128 * kd : 128 * (kd + 1)],
                        ident[:],
                    )

            def evict_xt():
                for kd in range(KD):
                    nc.scalar.copy(xt_sb[:, kd, :], xt_ps[kd][:])

            def mm(m):
                ps = pre_psum.tile([128, TC * BL], f32, name="prep", tag="prep")
                for kd in range(KD):
                    nc.tensor.matmul(
                        ps[:],
                        w_sb[:, kd, m * 128 : (m + 1) * 128],
                        xt_sb[:, kd, :],
                        start=(kd == 0),
                        stop=(kd == KD - 1),
                    )
                nc.scalar.activation(
                    gx_t[:, :, m, :],
                    ps[:].rearrange("p (t b) -> p t b", t=TC),
                    AF.Identity,
                    bias=b_sb[:, m : m + 1],
                )

            for j in range(4):
                units.append(lambda j=j: load(j))
            for j in range(4):
                units.append(lambda j=j: tr(j))
            units.append(evict_xt)
            for m in range(M3H):
                units.append(lambda m=m: mm(m))
            return units

        def emit_step(c, j):
            gx_t = gx_tiles[c]
            zr_ps = zr_psum.tile([128, 8 * BL], f32, name="zrp", tag="zrp")
            # r-gate matmuls first (m 4..7), then z (m 0..3), so the
            # r -> rh -> n chain overlaps the z matmuls on PE.
            for m in [4, 5, 6, 7, 0, 1, 2, 3]:
                for k in range(KH):
                    nc.tensor.matmul(
                        zr_ps[:, m * BL : (m + 1) * BL],
                        u_sb[:, k, m * 128 : (m + 1) * 128],
                        h_sb[:, k * BL : (k + 1) * BL],
                        start=(k == 0),
                        stop=(k == KH - 1),
                    )
            gr_sb = sb_pool.tile([128, 4 * BL], f32, name="gr", tag="gr")
            nc.vector.tensor_add(
                gr_sb[:].rearrange("p (m b) -> p m b", m=4),
                zr_ps[:, 4 * BL : 8 * BL].rearrange("p (m b) -> p m b", m=4),
                gx_t[:, j, 4:8, :],
            )
            r_sb = sb_pool.tile([128, 4 * BL], f32, name="r", tag="r")
            nc.scalar.activation(r_sb[:], gr_sb[:], AF.Sigmoid)
            rh_sb = sb_pool.tile([128, 4 * BL], f32, name="rh", tag="rh")
            nc.vector.tensor_mul(rh_sb[:], r_sb[:], h_sb[:])

            n_ps = n_psum.tile([128, 4 * BL], f32, name="npt", tag="np")
            for m in range(4):
                for k in range(KH):
                    nc.tensor.matmul(
                        n_ps[:, m * BL : (m + 1) * BL],
                        u_sb[:, k, 1024 + m * 128 : 1024 + (m + 1) * 128],
                        rh_sb[:, k * BL : (k + 1) * BL],
                        start=(k == 0),
                        stop=(k == KH - 1),
                    )

            gz_sb = sb_pool.tile([128, 4 * BL], f32, name="gz", tag="gz")
            nc.vector.tensor_add(
                gz_sb[:].rearrange("p (m b) -> p m b", m=4),
                zr_ps[:, 0 : 4 * BL].rearrange("p (m b) -> p m b", m=4),
                gx_t[:, j, 0:4, :],
            )
            z_sb = sb_pool.tile([128, 4 * BL], f32, name="z", tag="z")
            nc.scalar.activation(z_sb[:], gz_sb[:], AF.Sigmoid)

            gn_sb = sb_pool.tile([128, 4 * BL], f32, name="gn", tag="gn")
            nc.vector.tensor_add(
                gn_sb[:].rearrange("p (m b) -> p m b", m=4),
                n_ps[:].rearrange("p (m b) -> p m b", m=4),
                gx_t[:, j, 8:12, :],
            )
            n_sb = sb_pool.tile([128, 4 * BL], f32, name="n", tag="n")
            nc.scalar.activation(n_sb[:], gn_sb[:], AF.Tanh)

            # h = n + z*(h - n)
            tmp = sb_pool.tile([128, 4 * BL], f32, name="tmp", tag="tmp")
            nc.vector.tensor_sub(tmp[:], h_sb[:], n_sb[:])
            nc.vector.tensor_mul(tmp[:], z_sb[:], tmp[:])
            nc.vector.tensor_add(h_sb[:], n_sb[:], tmp[:])

        # ---- main emission ----
        # Chunk 0's precompute up front; chunk c+1's precompute interleaved
        # between chunk c's recurrence steps so it fills TensorE gaps.
        for u in make_units(0):
            u()
        for c in range(nchunk):
            pend = make_units(c + 1) if c + 1 < nchunk else []
            done = 0
            for j in range(TC):
                emit_step(c, j)
                want = (len(pend) * (j + 1) + TC - 1) // TC
                while done < min(want, len(pend)):
                    pend[done]()
                    done += 1
            while done < len(pend):
                pend[done]()
                done += 1

        # final dense head: y = h @ Wd + bd
        out_ps = n_psum.tile([BL, 1], f32, name="outp", tag="np")
        for k in range(KH):
            nc.tensor.matmul(
                out_ps[:],
                h_sb[:, k * BL : (k + 1) * BL],
                wd_sb[:, k : k + 1],
                start=(k == 0),
                stop=False,
            )
        nc.tensor.matmul(
            out_ps[:], ones_sb[0:1, :], bd_sb[0:1, :], start=False, stop=True
        )
        y_sb = sb_pool.tile([BL, 1], f32, name="y", tag="y")
        nc.vector.tensor_copy(y_sb[:], out_ps[:])
        nc.sync.dma_start(y_d[:], y_sb[:])

    nc.compile()
    return nc


def kernel(x, W, U, b, Wd, bd):
    from concourse.bass_utils import run_bass_kernel_spmd

    t_run = int(os.environ.get("GRU_T_RUN", T))
    key = t_run
    if key not in _CACHE:
        _CACHE[key] = _build(t_run)
    nc = _CACHE[key]

    x = np.ascontiguousarray(np.asarray(x, dtype=np.float32))
    W = np.ascontiguousarray(np.asarray(W, dtype=np.float32))
    U = np.ascontiguousarray(np.asarray(U, dtype=np.float32))
    b = np.ascontiguousarray(np.asarray(b, dtype=np.float32))
    Wd = np.ascontiguousarray(np.asarray(Wd, dtype=np.float32))
    bd = np.ascontiguousarray(np.asarray(bd, dtype=np.float32))

    in_maps = [
        {
            "x": np.ascontiguousarray(x[i * BL : (i + 1) * BL]),
            "W": W,
            "U": U,
            "b": b,
            "Wd": Wd,
            "bd": bd,
        }
        for i in range(NCORES)
    ]
    res = run_bass_kernel_spmd(
        nc,
        in_maps,
        core_ids=list(range(NCORES)),
        trace=os.environ.get("GRU_TRACE", "0") == "1",
    )
    out = np.concatenate([r["y"] for r in res.results], axis=0)
    if res.exec_time_ns is not None:
        print(f"HW exec time: {res.exec_time_ns} ns")
    return out



# revision 5
# speedup vs baseline: 4.0235x; 4.0235x over previous
"""Trainium2 Bass kernel for CustomGRUModel.

Reference computation (per batch row):
    gx = x @ W                       # [T, 3H] input projections (precomputed)
    per step t:
        gh_zr = h @ U[:, :2H]
        z = sigmoid(gxz + ghz + bz)
        r = sigmoid(gxr + ghr + br)
        n = tanh(gxn + (r*h) @ U[:, 2H:] + bn)
        h = z*h + (1-z)*n
    y = h_last @ Wd + bd

Sharding: data-parallel over batch, 32 rows per core on 8 cores. Weights
replicated. No collectives.

Per-core layout: everything transposed ("feature on partitions"):
  hT [H=512, B=32] stored as one SBUF tile [128, 4*32] (4 H-chunks packed in
  the free dim). Recurrent matmuls keep U as the stationary operand
  (lhsT = U k-tile slice [128, 128], fp32 exact) streaming hT chunks (N=32):
  output lands transposed [3H-chunk, B] in PSUM, which makes the gate
  elementwise work run on full 128 partitions.

The gx precompute runs chunked (16 steps at a time) in float32r (1 cyc/row at
N=512), interleaved between recurrence steps so it fills TensorE gaps. x is
transposed on-chip with PE transposes. The bias b is folded into the
PSUM->SBUF eviction of gx (ACT activation bias).
"""

import os

import numpy as np

B, T, D, H = 256, 512, 256, 512
NCORES = 8
BL = B // NCORES  # 32 batch rows per core
TC = 16  # timestep chunk for the gx precompute
KH = H // 128  # 4 k-tiles over H
KD = D // 128  # 2 k-tiles over D
M3H = 3 * H // 128  # 12 m-tiles over 3H

_CACHE = {}


def _build(t_run):
    from contextlib import ExitStack

    import concourse.bacc as bacc
    import concourse.bass as bass
    import concourse.tile as tile
    from concourse import masks, mybir

    dt = mybir.dt
    f32 = dt.float32
    f32r = dt.float32r
    bf16 = dt.bfloat16
    AF = mybir.ActivationFunctionType

    nchunk = t_run // TC

    nc = bacc.Bacc(
        "TRN2", target_bir_lowering=False, debug=False, num_devices=NCORES
    )
    x_d = nc.dram_tensor("x", [BL, T, D], f32, kind="ExternalInput")
    w_d = nc.dram_tensor("W", [D, 3 * H], f32, kind="ExternalInput")
    u_d = nc.dram_tensor("U", [H, 3 * H], f32, kind="ExternalInput")
    b_d = nc.dram_tensor("b", [3 * H], f32, kind="ExternalInput")
    wd_d = nc.dram_tensor("Wd", [H, 1], f32, kind="ExternalInput")
    bd_d = nc.dram_tensor("bd", [1], f32, kind="ExternalInput")
    y_d = nc.dram_tensor("y", [BL, 1], f32, kind="ExternalOutput")

    # chunked view of x: [chunk, tc, b, d]
    x_view = x_d.rearrange("b (c t) d -> c t b d", t=TC)

    with tile.TileContext(nc) as tc, ExitStack() as ctx:
        const = ctx.enter_context(tc.tile_pool(name="const", bufs=1))
        gx_pool = ctx.enter_context(tc.tile_pool(name="gx", bufs=2))
        xin_pool = ctx.enter_context(tc.tile_pool(name="xin", bufs=8))
        xt_pool = ctx.enter_context(tc.tile_pool(name="xt", bufs=2))
        sb_pool = ctx.enter_context(tc.tile_pool(name="sb", bufs=3))
        zr_psum = ctx.enter_context(
            tc.tile_pool(name="zrp", bufs=2, space=bass.MemorySpace.PSUM)
        )
        n_psum = ctx.enter_context(
            tc.tile_pool(name="np", bufs=2, space=bass.MemorySpace.PSUM)
        )
        pre_psum = ctx.enter_context(
            tc.tile_pool(name="prep", bufs=2, space=bass.MemorySpace.PSUM)
        )
        xt_psum = ctx.enter_context(
            tc.tile_pool(name="xtp", bufs=2, space=bass.MemorySpace.PSUM)
        )

        # ---- constants ----
        w_stage = const.tile([128, KD, 3 * H], f32)
        for k in range(KD):
            nc.sync.dma_start(w_stage[:, k, :], w_d[k * 128 : (k + 1) * 128, :])
        w_sb = const.tile([128, KD, 3 * H], f32r)
        for k in range(KD):
            nc.scalar.copy(w_sb[:, k, :], w_stage[:, k, :])
        u_stage = const.tile([128, KH, 3 * H], f32)
        for k in range(KH):
            nc.sync.dma_start(u_stage[:, k, :], u_d[k * 128 : (k + 1) * 128, :])
        u_sb = const.tile([128, KH, 3 * H], bf16)
        for k in range(KH):
            nc.scalar.copy(u_sb[:, k, :], u_stage[:, k, :])
        b_sb = const.tile([128, M3H], f32)
        nc.sync.dma_start(b_sb[:], b_d.rearrange("(m p) -> p m", p=128))
        wd_stage = const.tile([128, KH], f32)
        nc.sync.dma_start(wd_stage[:], wd_d.rearrange("(k p) o -> p (k o)", p=128))
        wd_sb = const.tile([128, KH], bf16)
        nc.scalar.copy(wd_sb[:], wd_stage[:])
        bd_stage = const.tile([1, 1], f32)
        nc.sync.dma_start(bd_stage[0:1, :], bd_d.rearrange("(o u) -> o u", u=1))
        bd_sb = const.tile([1, 1], bf16)
        nc.scalar.copy(bd_sb[0:1, :], bd_stage[0:1, :])
        ident = const.tile([128, 128], f32)
        masks.make_identity(nc, ident[:])
        ones_sb = const.tile([1, BL], bf16)
        nc.gpsimd.memset(ones_sb[0:1, :], 1.0)

        # persistent hidden state hT: [128, (k, b)] = [128, 4*32], bf16
        # (matmul moving operand; fp32 LDWEIGHTS+double-pumped MATMUL was the
        # bottleneck on HW)
        h_sb = const.tile([128, KH * BL], bf16)
        nc.gpsimd.memset(h_sb[:], 0.0)

        warm_ps = n_psum.tile([128, 128], f32, name="warm", tag="np")
        nc.tensor.transpose(warm_ps[:], ident[:], ident[:])

        gx_tiles = {}

        def make_units(c):
            """Emit-thunks for precomputing gx chunk c (16 steps)."""
            gx_t = gx_pool.tile([128, TC, M3H, BL], f32, name="gx", tag="gx")
            gx_tiles[c] = gx_t
            xins = []
            xt_sb = xt_pool.tile([128, KD, TC * BL], f32r, name="xt", tag="xt")
            xt_ps = {}
            units = []

            def load(j):
                t = xin_pool.tile([128, D], f32, name="xin", tag="xin")
                xins.append(t)
                nc.sync.dma_start(
                    t[:],
                    x_view[c, 4 * j : 4 * (j + 1)],
                )

            def tr(j):
                # transpose both d-chunks of xin row-block j
                for kd in range(KD):
                    if j == 0:
                        xt_ps[kd] = xt_psum.tile([128, TC * BL], f32, name="xtp", tag="xtp")
                    nc.tensor.transpose(
                        xt_ps[kd][:, 128 * j : 128 * (j + 1)],
                        xins[j][:, 128 * kd : 128 * (kd + 1)],
                        ident[:],
                    )

            def evict_xt():
                for kd in range(KD):
                    nc.scalar.copy(xt_sb[:, kd, :], xt_ps[kd][:])

            def mm(m):
                ps = pre_psum.tile([128, TC * BL], f32, name="prep", tag="prep")
                for kd in range(KD):
                    nc.tensor.matmul(
                        ps[:],
                        w_sb[:, kd, m * 128 : (m + 1) * 128],
                        xt_sb[:, kd, :],
                        start=(kd == 0),
                        stop=(kd == KD - 1),
                    )
                nc.scalar.activation(
                    gx_t[:, :, m, :],
                    ps[:].rearrange("p (t b) -> p t b", t=TC),
                    AF.Identity,
                    bias=b_sb[:, m : m + 1],
                )

            for j in range(4):
                units.append(lambda j=j: load(j))
            for j in range(4):
                units.append(lambda j=j: tr(j))
            units.append(evict_xt)
            for m in range(M3H):
                units.append(lambda m=m: mm(m))
            return units

        def emit_step(c, j):
            gx_t = gx_tiles[c]
            zr_ps = zr_psum.tile([128, 8 * BL], f32, name="zrp", tag="zrp")
            # r-gate matmuls first (m 4..7), then z (m 0..3), so the
            # r -> rh -> n chain overlaps the z matmuls on PE.
            for m in [4, 5, 6, 7, 0, 1, 2, 3]:
                for k in range(KH):
                    nc.tensor.matmul(
                        zr_ps[:, m * BL : (m + 1) * BL],
                        u_sb[:, k, m * 128 : (m + 1) * 128],
                        h_sb[:, k * BL : (k + 1) * BL],
                        start=(k == 0),
                        stop=(k == KH - 1),
                    )
            gr_sb = sb_pool.tile([128, 4 * BL], bf16, name="gr", tag="gr")
            nc.vector.tensor_add(
                gr_sb[:].rearrange("p (m b) -> p m b", m=4),
                zr_ps[:, 4 * BL : 8 * BL].rearrange("p (m b) -> p m b", m=4),
                gx_t[:, j, 4:8, :],
            )
            r_sb = sb_pool.tile([128, 4 * BL], bf16, name="r", tag="r")
            nc.scalar.activation(r_sb[:], gr_sb[:], AF.Sigmoid)
            rh_sb = sb_pool.tile([128, 4 * BL], bf16, name="rh", tag="rh")
            nc.vector.tensor_mul(rh_sb[:], r_sb[:], h_sb[:])

            n_ps = n_psum.tile([128, 4 * BL], f32, name="npt", tag="np")
            for m in range(4):
                for k in range(KH):
                    nc.tensor.matmul(
                        n_ps[:, m * BL : (m + 1) * BL],
                        u_sb[:, k, 1024 + m * 128 : 1024 + (m + 1) * 128],
                        rh_sb[:, k * BL : (k + 1) * BL],
                        start=(k == 0),
                        stop=(k == KH - 1),
                    )

            gz_sb = sb_pool.tile([128, 4 * BL], bf16, name="gz", tag="gz")
            nc.vector.tensor_add(
                gz_sb[:].rearrange("p (m b) -> p m b", m=4),
                zr_ps[:, 0 : 4 * BL].rearrange("p (m b) -> p m b", m=4),
                gx_t[:, j, 0:4, :],
            )
            z_sb = sb_pool.tile([128, 4 * BL], bf16, name="z", tag="z")
            nc.scalar.activation(z_sb[:], gz_sb[:], AF.Sigmoid)

            gn_sb = sb_pool.tile([128, 4 * BL], bf16, name="gn", tag="gn")
            nc.vector.tensor_add(
                gn_sb[:].rearrange("p (m b) -> p m b", m=4),
                n_ps[:].rearrange("p (m b) -> p m b", m=4),
                gx_t[:, j, 8:12, :],
            )
            n_sb = sb_pool.tile([128, 4 * BL], bf16, name="n", tag="n")
            nc.scalar.activation(n_sb[:], gn_sb[:], AF.Tanh)

            # h = n + z*(h - n)
            tmp = sb_pool.tile([128, 4 * BL], bf16, name="tmp", tag="tmp")
            nc.vector.tensor_sub(tmp[:], h_sb[:], n_sb[:])
            nc.vector.tensor_mul(tmp[:], z_sb[:], tmp[:])
            nc.vector.tensor_add(h_sb[:], n_sb[:], tmp[:])

        # ---- main emission ----
        # Chunk 0's precompute up front; chunk c+1's precompute interleaved
        # between chunk c's recurrence steps so it fills TensorE gaps.
        for u in make_units(0):
            u()
        for c in range(nchunk):
            pend = make_units(c + 1) if c + 1 < nchunk else []
            done = 0
            for j in range(TC):
                emit_step(c, j)
                want = (len(pend) * (j + 1) + TC - 1) // TC
                while done < min(want, len(pend)):
                    pend[done]()
                    done += 1
            while done < len(pend):
                pend[done]()
                done += 1

        # final dense head: y = h @ Wd + bd
        out_ps = n_psum.tile([BL, 1], f32, name="outp", tag="np")
        for k in range(KH):
            nc.tensor.matmul(
                out_ps[:],
                h_sb[:, k * BL : (k + 1) * BL],
                wd_sb[:, k : k + 1],
                start=(k == 0),
                stop=False,
            )
        nc.tensor.matmul(
            out_ps[:], ones_sb[0:1, :], bd_sb[0:1, :], start=False, stop=True
        )
        y_sb = sb_pool.tile([BL, 1], f32, name="y", tag="y")
        nc.vector.tensor_copy(y_sb[:], out_ps[:])
        nc.sync.dma_start(y_d[:], y_sb[:])

    nc.compile()
    return nc


def kernel(x, W, U, b, Wd, bd):
    from concourse.bass_utils import run_bass_kernel_spmd

    t_run = int(os.environ.get("GRU_T_RUN", T))
    key = t_run
    if key not in _CACHE:
        _CACHE[key] = _build(t_run)
    nc = _CACHE[key]

    x = np.ascontiguousarray(np.asarray(x, dtype=np.float32))
    W = np.ascontiguousarray(np.asarray(W, dtype=np.float32))
    U = np.ascontiguousarray(np.asarray(U, dtype=np.float32))
    b = np.ascontiguousarray(np.asarray(b, dtype=np.float32))
    Wd = np.ascontiguousarray(np.asarray(Wd, dtype=np.float32))
    bd = np.ascontiguousarray(np.asarray(bd, dtype=np.float32))

    in_maps = [
        {
            "x": np.ascontiguousarray(x[i * BL : (i + 1) * BL]),
            "W": W,
            "U": U,
            "b": b,
            "Wd": Wd,
            "bd": bd,
        }
        for i in range(NCORES)
    ]
    res = run_bass_kernel_spmd(
        nc,
        in_maps,
        core_ids=list(range(NCORES)),
        trace=os.environ.get("GRU_TRACE", "0") == "1",
    )
    out = np.concatenate([r["y"] for r in res.results], axis=0)
    if res.exec_time_ns is not None:
        print(f"HW exec time: {res.exec_time_ns} ns")
    return out



# revision 8
# speedup vs baseline: 5.0693x; 1.2599x over previous
"""Trainium2 Bass kernel for CustomGRUModel.

Reference computation (per batch row):
    gx = x @ W                       # [T, 3H] input projections (precomputed)
    per step t:
        gh_zr = h @ U[:, :2H]
        z = sigmoid(gxz + ghz + bz)
        r = sigmoid(gxr + ghr + br)
        n = tanh(gxn + (r*h) @ U[:, 2H:] + bn)
        h = z*h + (1-z)*n
    y = h_last @ Wd + bd

Sharding: data-parallel over batch, 32 rows per core on 8 cores. Weights
replicated. No collectives.

Per-core layout: everything transposed ("feature on partitions"):
  hT [H=512, B=32] stored as one SBUF tile [128, 4*32] (4 H-chunks packed in
  the free dim). Recurrent matmuls keep U as the stationary operand
  (lhsT = U k-tile slice [128, 128], fp32 exact) streaming hT chunks (N=32):
  output lands transposed [3H-chunk, B] in PSUM, which makes the gate
  elementwise work run on full 128 partitions.

The gx precompute runs chunked (16 steps at a time) in float32r (1 cyc/row at
N=512), interleaved between recurrence steps so it fills TensorE gaps. x is
transposed on-chip with PE transposes. The bias b is folded into the
PSUM->SBUF eviction of gx (ACT activation bias).
"""

import os

import numpy as np

B, T, D, H = 256, 512, 256, 512
NCORES = 8
BL = B // NCORES  # 32 batch rows per core
TC = 16  # timestep chunk for the gx precompute
KH = H // 128  # 4 k-tiles over H
KD = D // 128  # 2 k-tiles over D
M3H = 3 * H // 128  # 12 m-tiles over 3H

_CACHE = {}


def _build(t_run):
    from contextlib import ExitStack

    import concourse.bacc as bacc
    import concourse.bass as bass
    import concourse.tile as tile
    from concourse import masks, mybir

    dt = mybir.dt
    f32 = dt.float32
    f32r = dt.float32r
    bf16 = dt.bfloat16
    AF = mybir.ActivationFunctionType

    nchunk = t_run // TC

    nc = bacc.Bacc(
        "TRN2", target_bir_lowering=False, debug=False, num_devices=NCORES
    )
    x_d = nc.dram_tensor("x", [BL, T, D], f32, kind="ExternalInput")
    w_d = nc.dram_tensor("W", [D, 3 * H], f32, kind="ExternalInput")
    u_d = nc.dram_tensor("U", [H, 3 * H], f32, kind="ExternalInput")
    b_d = nc.dram_tensor("b", [3 * H], f32, kind="ExternalInput")
    wd_d = nc.dram_tensor("Wd", [H, 1], f32, kind="ExternalInput")
    bd_d = nc.dram_tensor("bd", [1], f32, kind="ExternalInput")
    y_d = nc.dram_tensor("y", [BL, 1], f32, kind="ExternalOutput")

    # chunked view of x: [chunk, tc, b, d]
    x_view = x_d.rearrange("b (c t) d -> c t b d", t=TC)

    with tile.TileContext(nc) as tc, ExitStack() as ctx:
        const = ctx.enter_context(tc.tile_pool(name="const", bufs=1))
        gx_pool = ctx.enter_context(tc.tile_pool(name="gx", bufs=2))
        xin_pool = ctx.enter_context(tc.tile_pool(name="xin", bufs=8))
        xt_pool = ctx.enter_context(tc.tile_pool(name="xt", bufs=2))
        sb_pool = ctx.enter_context(tc.tile_pool(name="sb", bufs=3))
        zr_psum = ctx.enter_context(
            tc.tile_pool(name="zrp", bufs=2, space=bass.MemorySpace.PSUM)
        )
        n_psum = ctx.enter_context(
            tc.tile_pool(name="np", bufs=2, space=bass.MemorySpace.PSUM)
        )
        pre_psum = ctx.enter_context(
            tc.tile_pool(name="prep", bufs=2, space=bass.MemorySpace.PSUM)
        )
        xt_psum = ctx.enter_context(
            tc.tile_pool(name="xtp", bufs=2, space=bass.MemorySpace.PSUM)
        )

        # ---- constants ----
        w_stage = const.tile([128, KD, 3 * H], f32)
        for k in range(KD):
            nc.sync.dma_start(w_stage[:, k, :], w_d[k * 128 : (k + 1) * 128, :])
        w_sb = const.tile([128, KD, 3 * H], bf16)
        for k in range(KD):
            nc.scalar.copy(w_sb[:, k, :], w_stage[:, k, :])
        u_stage = const.tile([128, KH, 3 * H], f32)
        for k in range(KH):
            nc.sync.dma_start(u_stage[:, k, :], u_d[k * 128 : (k + 1) * 128, :])
        u_sb = const.tile([128, KH, 3 * H], bf16)
        for k in range(KH):
            nc.scalar.copy(u_sb[:, k, :], u_stage[:, k, :])
        b_sb = const.tile([128, M3H], f32)
        nc.sync.dma_start(b_sb[:], b_d.rearrange("(m p) -> p m", p=128))
        wd_stage = const.tile([128, KH], f32)
        nc.sync.dma_start(wd_stage[:], wd_d.rearrange("(k p) o -> p (k o)", p=128))
        wd_sb = const.tile([128, KH], bf16)
        nc.scalar.copy(wd_sb[:], wd_stage[:])
        bd_stage = const.tile([1, 1], f32)
        nc.sync.dma_start(bd_stage[0:1, :], bd_d.rearrange("(o u) -> o u", u=1))
        bd_sb = const.tile([1, 1], bf16)
        nc.scalar.copy(bd_sb[0:1, :], bd_stage[0:1, :])
        ident = const.tile([128, 128], f32)
        masks.make_identity(nc, ident[:])
        ones_sb = const.tile([1, BL], bf16)
        nc.gpsimd.memset(ones_sb[0:1, :], 1.0)

        # persistent hidden state hT: [128, (k, b)] = [128, 4*32], bf16
        # (matmul moving operand; fp32 LDWEIGHTS+double-pumped MATMUL was the
        # bottleneck on HW)
        h_sb = const.tile([128, KH * BL], bf16)
        nc.gpsimd.memset(h_sb[:], 0.0)

        warm_ps = n_psum.tile([128, 128], f32, name="warm", tag="np")
        nc.tensor.transpose(warm_ps[:], ident[:], ident[:])

        gx_tiles = {}

        def make_units(c):
            """Emit-thunks for precomputing gx chunk c (16 steps)."""
            gx_t = gx_pool.tile([128, TC, M3H, BL], f32, name="gx", tag="gx")
            gx_tiles[c] = gx_t
            xins = []
            xt_sb = xt_pool.tile([128, KD, TC * BL], bf16, name="xt", tag="xt")
            xt_ps = {}
            units = []

            def load(j):
                t = xin_pool.tile([128, D], f32, name="xin", tag="xin")
                xins.append(t)
                nc.sync.dma_start(
                    t[:],
                    x_view[c, 4 * j : 4 * (j + 1)],
                )

            def tr(j):
                # transpose both d-chunks of xin row-block j
                for kd in range(KD):
                    if j == 0:
                        xt_ps[kd] = xt_psum.tile([128, TC * BL], f32, name="xtp", tag="xtp")
                    nc.tensor.transpose(
                        xt_ps[kd][:, 128 * j : 128 * (j + 1)],
                        xins[j][:, 128 * kd : 128 * (kd + 1)],
                        ident[:],
                    )

            def evict_xt():
                for kd in range(KD):
                    nc.scalar.copy(xt_sb[:, kd, :], xt_ps[kd][:])

            def mm(m):
                ps = pre_psum.tile([128, TC * BL], f32, name="prep", tag="prep")
                for kd in range(KD):
                    nc.tensor.matmul(
                        ps[:],
                        w_sb[:, kd, m * 128 : (m + 1) * 128],
                        xt_sb[:, kd, :],
                        start=(kd == 0),
                        stop=(kd == KD - 1),
                    )
                nc.scalar.activation(
                    gx_t[:, :, m, :],
                    ps[:].rearrange("p (t b) -> p t b", t=TC),
                    AF.Identity,
                    bias=b_sb[:, m : m + 1],
                )

            for j in range(4):
                units.append(lambda j=j: load(j))
            for j in range(4):
                units.append(lambda j=j: tr(j))
            units.append(evict_xt)
            for m in range(M3H):
                units.append(lambda m=m: mm(m))
            return units

        def emit_step(c, j):
            gx_t = gx_tiles[c]
            # PSUM pre-init: copy gx for this step into the accumulators, so
            # the recurrent matmuls (start=False) accumulate on top and the
            # gate ADDs vanish from the critical path. The copies depend only
            # on gx (ready a chunk ahead) and the recycled PSUM buffer, so
            # they run during the previous step's tail.
            zr_ps = zr_psum.tile([128, 8 * BL], f32, name="zrp", tag="zrp")
            nc.vector.tensor_copy(
                zr_ps[:].rearrange("p (m b) -> p m b", m=8), gx_t[:, j, 0:8, :]
            )
            n_ps = n_psum.tile([128, 4 * BL], f32, name="npt", tag="np")
            nc.scalar.copy(
                n_ps[:].rearrange("p (m b) -> p m b", m=4), gx_t[:, j, 8:12, :]
            )

            # r-gate matmuls, grouped per m-tile (m=4+kk computes the r chunk
            # for H-rows kk) so each rh chunk is ready as early as possible.
            for kk in range(KH):
                m = 4 + kk
                for k in range(KH):
                    nc.tensor.matmul(
                        zr_ps[:, m * BL : (m + 1) * BL],
                        u_sb[:, k, m * 128 : (m + 1) * 128],
                        h_sb[:, k * BL : (k + 1) * BL],
                        start=False,
                        stop=(k == KH - 1),
                        skip_group_check=True,
                    )
            r_sb = sb_pool.tile([128, 4 * BL], bf16, name="r", tag="r")
            rh_sb = sb_pool.tile([128, 4 * BL], bf16, name="rh", tag="rh")
            for kk in range(KH):
                nc.scalar.activation(
                    r_sb[:, kk * BL : (kk + 1) * BL],
                    zr_ps[:, (4 + kk) * BL : (5 + kk) * BL],
                    AF.Sigmoid,
                )
                nc.vector.tensor_mul(
                    rh_sb[:, kk * BL : (kk + 1) * BL],
                    r_sb[:, kk * BL : (kk + 1) * BL],
                    h_sb[:, kk * BL : (kk + 1) * BL],
                )

            # n-gate matmuls k-outer (layer k consumes rh chunk k), with the
            # z-gate matmuls slotted between layers to fill PE time while
            # later rh chunks are still being produced.
            def n_layer(k):
                for m in range(4):
                    nc.tensor.matmul(
                        n_ps[:, m * BL : (m + 1) * BL],
                        u_sb[:, k, 1024 + m * 128 : 1024 + (m + 1) * 128],
                        rh_sb[:, k * BL : (k + 1) * BL],
                        start=False,
                        stop=(k == KH - 1),
                        skip_group_check=True,
                    )

            def z_block(ms):
                for m in ms:
                    for k in range(KH):
                        nc.tensor.matmul(
                            zr_ps[:, m * BL : (m + 1) * BL],
                            u_sb[:, k, m * 128 : (m + 1) * 128],
                            h_sb[:, k * BL : (k + 1) * BL],
                            start=False,
                            stop=(k == KH - 1),
                            skip_group_check=True,
                        )

            n_layer(0)
            z_block([0, 1])
            n_layer(1)
            z_block([2, 3])
            n_layer(2)
            n_layer(3)

            n_sb = sb_pool.tile([128, 4 * BL], bf16, name="n", tag="n")
            nc.scalar.activation(n_sb[:], n_ps[:], AF.Tanh)
            z_sb = sb_pool.tile([128, 4 * BL], bf16, name="z", tag="z")
            nc.scalar.activation(z_sb[:], zr_ps[:, 0 : 4 * BL], AF.Sigmoid)

            # h = n + z*(h - n)
            tmp = sb_pool.tile([128, 4 * BL], bf16, name="tmp", tag="tmp")
            nc.vector.tensor_sub(tmp[:], h_sb[:], n_sb[:])
            nc.vector.tensor_mul(tmp[:], z_sb[:], tmp[:])
            nc.vector.tensor_add(h_sb[:], n_sb[:], tmp[:])

        # ---- main emission ----
        # Chunk 0's precompute up front; chunk c+1's precompute interleaved
        # between chunk c's recurrence steps so it fills TensorE gaps.
        for u in make_units(0):
            u()
        for c in range(nchunk):
            pend = make_units(c + 1) if c + 1 < nchunk else []
            done = 0
            for j in range(TC):
                emit_step(c, j)
                want = (len(pend) * (j + 1) + TC - 1) // TC
                while done < min(want, len(pend)):
                    pend[done]()
                    done += 1
            while done < len(pend):
                pend[done]()
                done += 1

        # final dense head: y = h @ Wd + bd
        out_ps = n_psum.tile([BL, 1], f32, name="outp", tag="np")
        for k in range(KH):
            nc.tensor.matmul(
                out_ps[:],
                h_sb[:, k * BL : (k + 1) * BL],
                wd_sb[:, k : k + 1],
                start=(k == 0),
                stop=False,
            )
        nc.tensor.matmul(
            out_ps[:], ones_sb[0:1, :], bd_sb[0:1, :], start=False, stop=True
        )
        y_sb = sb_pool.tile([BL, 1], f32, name="y", tag="y")
        nc.vector.tensor_copy(y_sb[:], out_ps[:])
        nc.sync.dma_start(y_d[:], y_sb[:])

    nc.compile()
    return nc


def kernel(x, W, U, b, Wd, bd):
    from concourse.bass_utils import run_bass_kernel_spmd

    t_run = int(os.environ.get("GRU_T_RUN", T))
    key = t_run
    if key not in _CACHE:
        _CACHE[key] = _build(t_run)
    nc = _CACHE[key]

    x = np.ascontiguousarray(np.asarray(x, dtype=np.float32))
    W = np.ascontiguousarray(np.asarray(W, dtype=np.float32))
    U = np.ascontiguousarray(np.asarray(U, dtype=np.float32))
    b = np.ascontiguousarray(np.asarray(b, dtype=np.float32))
    Wd = np.ascontiguousarray(np.asarray(Wd, dtype=np.float32))
    bd = np.ascontiguousarray(np.asarray(bd, dtype=np.float32))

    in_maps = [
        {
            "x": np.ascontiguousarray(x[i * BL : (i + 1) * BL]),
            "W": W,
            "U": U,
            "b": b,
            "Wd": Wd,
            "bd": bd,
        }
        for i in range(NCORES)
    ]
    res = run_bass_kernel_spmd(
        nc,
        in_maps,
        core_ids=list(range(NCORES)),
        trace=os.environ.get("GRU_TRACE", "0") == "1",
    )
    out = np.concatenate([r["y"] for r in res.results], axis=0)
    if res.exec_time_ns is not None:
        print(f"HW exec time: {res.exec_time_ns} ns")
    return out



# revision 12
# speedup vs baseline: 5.5737x; 1.0995x over previous
"""Trainium2 Bass kernel for CustomGRUModel.

Reference computation (per batch row):
    gx = x @ W                       # [T, 3H] input projections (precomputed)
    per step t:
        gh_zr = h @ U[:, :2H]
        z = sigmoid(gxz + ghz + bz)
        r = sigmoid(gxr + ghr + br)
        n = tanh(gxn + (r*h) @ U[:, 2H:] + bn)
        h = z*h + (1-z)*n
    y = h_last @ Wd + bd

Sharding: data-parallel over batch, 32 rows per core on 8 cores. Weights
replicated. No collectives.

Per-core layout: everything transposed ("feature on partitions"):
  hT [H=512, B=32] stored as one SBUF tile [128, 4*32] (4 H-chunks packed in
  the free dim). Recurrent matmuls keep U as the stationary operand
  (lhsT = U k-tile slice [128, 128], fp32 exact) streaming hT chunks (N=32):
  output lands transposed [3H-chunk, B] in PSUM, which makes the gate
  elementwise work run on full 128 partitions.

The gx precompute runs chunked (16 steps at a time) in float32r (1 cyc/row at
N=512), interleaved between recurrence steps so it fills TensorE gaps. x is
transposed on-chip with PE transposes. The bias b is folded into the
PSUM->SBUF eviction of gx (ACT activation bias).
"""

import os

import numpy as np

B, T, D, H = 256, 512, 256, 512
NCORES = 8
BL = B // NCORES  # 32 batch rows per core
TC = 16  # timestep chunk for the gx precompute
KH = H // 128  # 4 k-tiles over H
KD = D // 128  # 2 k-tiles over D
M3H = 3 * H // 128  # 12 m-tiles over 3H

_CACHE = {}


def _build(t_run):
    from contextlib import ExitStack

    import concourse.bacc as bacc
    import concourse.bass as bass
    import concourse.tile as tile
    from concourse import masks, mybir

    dt = mybir.dt
    f32 = dt.float32
    f32r = dt.float32r
    bf16 = dt.bfloat16
    AF = mybir.ActivationFunctionType

    nchunk = t_run // TC

    nc = bacc.Bacc(
        "TRN2", target_bir_lowering=False, debug=False, num_devices=NCORES
    )
    x_d = nc.dram_tensor("x", [BL, T, D], f32, kind="ExternalInput")
    w_d = nc.dram_tensor("W", [D, 3 * H], f32, kind="ExternalInput")
    u_d = nc.dram_tensor("U", [H, 3 * H], f32, kind="ExternalInput")
    b_d = nc.dram_tensor("b", [3 * H], f32, kind="ExternalInput")
    wd_d = nc.dram_tensor("Wd", [H, 1], f32, kind="ExternalInput")
    bd_d = nc.dram_tensor("bd", [1], f32, kind="ExternalInput")
    y_d = nc.dram_tensor("y", [BL, 1], f32, kind="ExternalOutput")

    # chunked view of x: [chunk, tc, b, d]
    x_view = x_d.rearrange("b (c t) d -> c t b d", t=TC)

    with tile.TileContext(nc) as tc, ExitStack() as ctx:
        const = ctx.enter_context(tc.tile_pool(name="const", bufs=1))
        gx_pool = ctx.enter_context(tc.tile_pool(name="gx", bufs=2))
        xin_pool = ctx.enter_context(tc.tile_pool(name="xin", bufs=8))
        xt_pool = ctx.enter_context(tc.tile_pool(name="xt", bufs=2))
        sb_pool = ctx.enter_context(tc.tile_pool(name="sb", bufs=3))
        zr_psum = ctx.enter_context(
            tc.tile_pool(name="zrp", bufs=2, space=bass.MemorySpace.PSUM)
        )
        n_psum = ctx.enter_context(
            tc.tile_pool(name="np", bufs=2, space=bass.MemorySpace.PSUM)
        )
        pre_psum = ctx.enter_context(
            tc.tile_pool(name="prep", bufs=2, space=bass.MemorySpace.PSUM)
        )
        xt_psum = ctx.enter_context(
            tc.tile_pool(name="xtp", bufs=2, space=bass.MemorySpace.PSUM)
        )

        # ---- constants ----
        w_stage = const.tile([128, KD, 3 * H], f32)
        for k in range(KD):
            nc.sync.dma_start(w_stage[:, k, :], w_d[k * 128 : (k + 1) * 128, :])
        w_sb = const.tile([128, KD, 3 * H], bf16)
        for k in range(KD):
            nc.scalar.copy(w_sb[:, k, :], w_stage[:, k, :])
        u_stage = const.tile([128, KH, 3 * H], f32)
        for k in range(KH):
            nc.sync.dma_start(u_stage[:, k, :], u_d[k * 128 : (k + 1) * 128, :])
        u_sb = const.tile([128, KH, 3 * H], bf16)
        for k in range(KH):
            nc.scalar.copy(u_sb[:, k, :], u_stage[:, k, :])
        b_sb = const.tile([128, M3H], f32)
        nc.sync.dma_start(b_sb[:], b_d.rearrange("(m p) -> p m", p=128))
        wd_stage = const.tile([128, KH], f32)
        nc.sync.dma_start(wd_stage[:], wd_d.rearrange("(k p) o -> p (k o)", p=128))
        wd_sb = const.tile([128, KH], bf16)
        nc.scalar.copy(wd_sb[:], wd_stage[:])
        bd_stage = const.tile([1, 1], f32)
        nc.sync.dma_start(bd_stage[0:1, :], bd_d.rearrange("(o u) -> o u", u=1))
        bd_sb = const.tile([1, 1], bf16)
        nc.scalar.copy(bd_sb[0:1, :], bd_stage[0:1, :])
        ident = const.tile([128, 128], f32)
        masks.make_identity(nc, ident[:])
        identb = const.tile([128, 128], bf16)
        nc.scalar.copy(identb[:], ident[:])
        ones_sb = const.tile([1, BL], bf16)
        nc.gpsimd.memset(ones_sb[0:1, :], 1.0)

        # persistent hidden state hT: [128, (k, b)] = [128, 4*32], bf16
        # (matmul moving operand; fp32 LDWEIGHTS+double-pumped MATMUL was the
        # bottleneck on HW)
        h_sb = const.tile([128, KH * BL], bf16)
        nc.gpsimd.memset(h_sb[:], 0.0)

        warm_ps = n_psum.tile([128, 128], f32, name="warm", tag="np")
        nc.tensor.transpose(warm_ps[:], ident[:], ident[:])

        gx_tiles = {}

        def make_units(c):
            """Emit-thunks for precomputing gx chunk c (16 steps)."""
            gx_t = gx_pool.tile([128, TC, M3H, BL], bf16, name="gx", tag="gx")
            gx_tiles[c] = gx_t
            xins = []
            xt_sb = xt_pool.tile([128, KD, TC * BL], bf16, name="xt", tag="xt")
            xt_ps = {}
            units = []

            def load(j):
                t = xin_pool.tile([128, D], f32, name="xin", tag="xin")
                xins.append(t)
                nc.sync.dma_start(
                    t[:],
                    x_view[c, 4 * j : 4 * (j + 1)],
                )

            def tr(j):
                # transpose both d-chunks of xin row-block j
                for kd in range(KD):
                    if j == 0:
                        xt_ps[kd] = xt_psum.tile([128, TC * BL], f32, name="xtp", tag="xtp")
                    nc.tensor.transpose(
                        xt_ps[kd][:, 128 * j : 128 * (j + 1)],
                        xins[j][:, 128 * kd : 128 * (kd + 1)],
                        ident[:],
                    )

            def evict_xt():
                for kd in range(KD):
                    nc.scalar.copy(xt_sb[:, kd, :], xt_ps[kd][:])

            def mm(m):
                ps = pre_psum.tile([128, TC * BL], f32, name="prep", tag="prep")
                for kd in range(KD):
                    nc.tensor.matmul(
                        ps[:],
                        w_sb[:, kd, m * 128 : (m + 1) * 128],
                        xt_sb[:, kd, :],
                        start=(kd == 0),
                        stop=(kd == KD - 1),
                    )
                # b folded into the eviction; alternate engines to balance
                # ACT/DVE load.
                if m % 2 == 0:
                    nc.scalar.activation(
                        gx_t[:, :, m, :],
                        ps[:].rearrange("p (t b) -> p t b", t=TC),
                        AF.Identity,
                        bias=b_sb[:, m : m + 1],
                    )
                else:
                    nc.vector.tensor_scalar(
                        gx_t[:, :, m, :],
                        ps[:].rearrange("p (t b) -> p t b", t=TC),
                        b_sb[:, m : m + 1],
                        None,
                        mybir.AluOpType.add,
                    )

            for j in range(4):
                units.append(lambda j=j: load(j))
            for j in range(4):
                units.append(lambda j=j: tr(j))
            units.append(evict_xt)
            for m in range(M3H):
                units.append(lambda m=m: mm(m))
            return units

        def emit_step(c, j):
            gx_t = gx_tiles[c]
            # PSUM pre-init on the PE itself: an identity-stationary matmul
            # streams the step's gx slice into the accumulator (start=True),
            # then the recurrent matmuls (start=False) accumulate on top.
            # Keeps the gate ADDs entirely off ACT/DVE.
            zr_ps = zr_psum.tile([128, 8 * BL], f32, name="zrp", tag="zrp")
            nc.tensor.matmul(
                zr_ps[:],
                identb[:],
                gx_t[:, j, 0:8, :],
                start=True,
                stop=False,
                skip_group_check=True,
            )
            n_ps = n_psum.tile([128, 4 * BL], f32, name="npt", tag="np")
            nc.tensor.matmul(
                n_ps[:],
                identb[:],
                gx_t[:, j, 8:12, :],
                start=True,
                stop=False,
                skip_group_check=True,
            )

            # r-gate matmuls first (m=4+kk computes the r chunk for H-rows
            # kk), then z; the r -> rh -> n chain overlaps the z matmuls.
            for m in [4, 5, 6, 7, 0, 1, 2, 3]:
                for k in range(KH):
                    nc.tensor.matmul(
                        zr_ps[:, m * BL : (m + 1) * BL],
                        u_sb[:, k, m * 128 : (m + 1) * 128],
                        h_sb[:, k * BL : (k + 1) * BL],
                        start=False,
                        stop=(m in (3, 7) and k == KH - 1),
                        skip_group_check=True,
                    )
            # r/rh in two half-chunks: first half unblocks the n-gate k=0/1
            # layers early, without paying 4x per-op overhead.
            r_sb = sb_pool.tile([128, 4 * BL], bf16, name="r", tag="r")
            rh_sb = sb_pool.tile([128, 4 * BL], bf16, name="rh", tag="rh")
            for half in range(2):
                sl = slice(half * 2 * BL, (half + 1) * 2 * BL)
                nc.scalar.activation(
                    r_sb[:, sl],
                    zr_ps[:, (4 + half * 2) * BL : (6 + half * 2) * BL],
                    AF.Sigmoid,
                )
                nc.vector.tensor_mul(rh_sb[:, sl], r_sb[:, sl], h_sb[:, sl])

            # n-gate matmuls k-outer: layer k consumes rh chunk k.
            for k in range(KH):
                for m in range(4):
                    nc.tensor.matmul(
                        n_ps[:, m * BL : (m + 1) * BL],
                        u_sb[:, k, 1024 + m * 128 : 1024 + (m + 1) * 128],
                        rh_sb[:, k * BL : (k + 1) * BL],
                        start=False,
                        stop=(k == KH - 1),
                        skip_group_check=True,
                    )

            z_sb = sb_pool.tile([128, 4 * BL], bf16, name="z", tag="z")
            nc.scalar.activation(z_sb[:], zr_ps[:, 0 : 4 * BL], AF.Sigmoid)
            n_sb = sb_pool.tile([128, 4 * BL], bf16, name="n", tag="n")
            nc.scalar.activation(n_sb[:], n_ps[:], AF.Tanh)

            # h = n + z*(h - n)
            tmp = sb_pool.tile([128, 4 * BL], bf16, name="tmp", tag="tmp")
            nc.vector.tensor_sub(tmp[:], h_sb[:], n_sb[:])
            nc.vector.tensor_mul(tmp[:], z_sb[:], tmp[:])
            nc.vector.tensor_add(h_sb[:], n_sb[:], tmp[:])

        # ---- main emission ----
        # Chunk 0's precompute up front; chunk c+1's precompute interleaved
        # between chunk c's recurrence steps so it fills TensorE gaps.
        for u in make_units(0):
            u()
        for c in range(nchunk):
            pend = make_units(c + 1) if c + 1 < nchunk else []
            done = 0
            for j in range(TC):
                emit_step(c, j)
                want = (len(pend) * (j + 1) + TC - 1) // TC
                while done < min(want, len(pend)):
                    pend[done]()
                    done += 1
            while done < len(pend):
                pend[done]()
                done += 1

        # final dense head: y = h @ Wd + bd
        out_ps = n_psum.tile([BL, 1], f32, name="outp", tag="np")
        for k in range(KH):
            nc.tensor.matmul(
                out_ps[:],
                h_sb[:, k * BL : (k + 1) * BL],
                wd_sb[:, k : k + 1],
                start=(k == 0),
                stop=False,
            )
        nc.tensor.matmul(
            out_ps[:], ones_sb[0:1, :], bd_sb[0:1, :], start=False, stop=True
        )
        y_sb = sb_pool.tile([BL, 1], f32, name="y", tag="y")
        nc.vector.tensor_copy(y_sb[:], out_ps[:])
        nc.sync.dma_start(y_d[:], y_sb[:])

    nc.compile()
    return nc


def kernel(x, W, U, b, Wd, bd):
    from concourse.bass_utils import run_bass_kernel_spmd

    t_run = int(os.environ.get("GRU_T_RUN", T))
    key = t_run
    if key not in _CACHE:
        _CACHE[key] = _build(t_run)
    nc = _CACHE[key]

    x = np.ascontiguousarray(np.asarray(x, dtype=np.float32))
    W = np.ascontiguousarray(np.asarray(W, dtype=np.float32))
    U = np.ascontiguousarray(np.asarray(U, dtype=np.float32))
    b = np.ascontiguousarray(np.asarray(b, dtype=np.float32))
    Wd = np.ascontiguousarray(np.asarray(Wd, dtype=np.float32))
    bd = np.ascontiguousarray(np.asarray(bd, dtype=np.float32))

    in_maps = [
        {
            "x": np.ascontiguousarray(x[i * BL : (i + 1) * BL]),
            "W": W,
            "U": U,
            "b": b,
            "Wd": Wd,
            "bd": bd,
        }
        for i in range(NCORES)
    ]
    res = run_bass_kernel_spmd(
        nc,
        in_maps,
        core_ids=list(range(NCORES)),
        trace=os.environ.get("GRU_TRACE", "0") == "1",
    )
    out = np.concatenate([r["y"] for r in res.results], axis=0)
    if res.exec_time_ns is not None:
        print(f"HW exec time: {res.exec_time_ns} ns")
    return out



# revision 16
# speedup vs baseline: 5.8614x; 1.0516x over previous
"""Trainium2 Bass kernel for CustomGRUModel.

Reference computation (per batch row):
    gx = x @ W                       # [T, 3H] input projections (precomputed)
    per step t:
        gh_zr = h @ U[:, :2H]
        z = sigmoid(gxz + ghz + bz)
        r = sigmoid(gxr + ghr + br)
        n = tanh(gxn + (r*h) @ U[:, 2H:] + bn)
        h = z*h + (1-z)*n
    y = h_last @ Wd + bd

Sharding: data-parallel over batch, 32 rows per core on 8 cores. Weights
replicated. No collectives.

Per-core layout: everything transposed ("feature on partitions"):
  hT [H=512, B=32] stored as one SBUF tile [128, 4*32] (4 H-chunks packed in
  the free dim). Recurrent matmuls keep U as the stationary operand
  (lhsT = U k-tile slice [128, 128], fp32 exact) streaming hT chunks (N=32):
  output lands transposed [3H-chunk, B] in PSUM, which makes the gate
  elementwise work run on full 128 partitions.

The gx precompute runs chunked (16 steps at a time) in float32r (1 cyc/row at
N=512), interleaved between recurrence steps so it fills TensorE gaps. x is
transposed on-chip with PE transposes. The bias b is folded into the
PSUM->SBUF eviction of gx (ACT activation bias).
"""

import os

import numpy as np

B, T, D, H = 256, 512, 256, 512
NCORES = 8
BL = B // NCORES  # 32 batch rows per core
TC = 16  # timestep chunk for the gx precompute
KH = H // 128  # 4 k-tiles over H
KD = D // 128  # 2 k-tiles over D
M3H = 3 * H // 128  # 12 m-tiles over 3H

_CACHE = {}


def _build(t_run):
    from contextlib import ExitStack

    import concourse.bacc as bacc
    import concourse.bass as bass
    import concourse.tile as tile
    from concourse import masks, mybir

    dt = mybir.dt
    f32 = dt.float32
    f32r = dt.float32r
    bf16 = dt.bfloat16
    AF = mybir.ActivationFunctionType

    nchunk = t_run // TC

    nc = bacc.Bacc(
        "TRN2", target_bir_lowering=False, debug=False, num_devices=NCORES
    )
    x_d = nc.dram_tensor("x", [BL, T, D], f32, kind="ExternalInput")
    w_d = nc.dram_tensor("W", [D, 3 * H], f32, kind="ExternalInput")
    u_d = nc.dram_tensor("U", [H, 3 * H], f32, kind="ExternalInput")
    b_d = nc.dram_tensor("b", [3 * H], f32, kind="ExternalInput")
    wd_d = nc.dram_tensor("Wd", [H, 1], f32, kind="ExternalInput")
    bd_d = nc.dram_tensor("bd", [1], f32, kind="ExternalInput")
    y_d = nc.dram_tensor("y", [BL, 1], f32, kind="ExternalOutput")

    # chunked view of x: [chunk, tc, b, d]
    x_view = x_d.rearrange("b (c t) d -> c t b d", t=TC)

    with tile.TileContext(nc) as tc, ExitStack() as ctx:
        const = ctx.enter_context(tc.tile_pool(name="const", bufs=1))
        gx_pool = ctx.enter_context(tc.tile_pool(name="gx", bufs=2))
        xin_pool = ctx.enter_context(tc.tile_pool(name="xin", bufs=8))
        xt_pool = ctx.enter_context(tc.tile_pool(name="xt", bufs=2))
        sb_pool = ctx.enter_context(tc.tile_pool(name="sb", bufs=3))
        r45_psum = ctx.enter_context(
            tc.tile_pool(name="rp45", bufs=1, space=bass.MemorySpace.PSUM)
        )
        r67_psum = ctx.enter_context(
            tc.tile_pool(name="rp67", bufs=1, space=bass.MemorySpace.PSUM)
        )
        z_psum = ctx.enter_context(
            tc.tile_pool(name="zp", bufs=1, space=bass.MemorySpace.PSUM)
        )
        n_psum = ctx.enter_context(
            tc.tile_pool(name="np", bufs=1, space=bass.MemorySpace.PSUM)
        )
        pre_psum = ctx.enter_context(
            tc.tile_pool(name="prep", bufs=2, space=bass.MemorySpace.PSUM)
        )
        xt_psum = ctx.enter_context(
            tc.tile_pool(name="xtp", bufs=2, space=bass.MemorySpace.PSUM)
        )

        # ---- constants ----
        w_stage = const.tile([128, KD, 3 * H], f32)
        for k in range(KD):
            nc.sync.dma_start(w_stage[:, k, :], w_d[k * 128 : (k + 1) * 128, :])
        w_sb = const.tile([128, KD, 3 * H], bf16)
        for k in range(KD):
            nc.scalar.copy(w_sb[:, k, :], w_stage[:, k, :])
        u_stage = const.tile([128, KH, 3 * H], f32)
        for k in range(KH):
            nc.sync.dma_start(u_stage[:, k, :], u_d[k * 128 : (k + 1) * 128, :])
        u_sb = const.tile([128, KH, 3 * H], bf16)
        for k in range(KH):
            nc.scalar.copy(u_sb[:, k, :], u_stage[:, k, :])
        b_sb = const.tile([128, M3H], f32)
        nc.sync.dma_start(b_sb[:], b_d.rearrange("(m p) -> p m", p=128))
        wd_stage = const.tile([128, KH], f32)
        nc.sync.dma_start(wd_stage[:], wd_d.rearrange("(k p) o -> p (k o)", p=128))
        wd_sb = const.tile([128, KH], bf16)
        nc.scalar.copy(wd_sb[:], wd_stage[:])
        bd_stage = const.tile([1, 1], f32)
        nc.sync.dma_start(bd_stage[0:1, :], bd_d.rearrange("(o u) -> o u", u=1))
        bd_sb = const.tile([1, 1], bf16)
        nc.scalar.copy(bd_sb[0:1, :], bd_stage[0:1, :])
        ident = const.tile([128, 128], f32)
        masks.make_identity(nc, ident[:])
        identb = const.tile([128, 128], bf16)
        nc.scalar.copy(identb[:], ident[:])
        ones_sb = const.tile([1, BL], bf16)
        nc.gpsimd.memset(ones_sb[0:1, :], 1.0)

        # persistent hidden state hT: [128, (k, b)] = [128, 4*32], bf16
        # (matmul moving operand; fp32 LDWEIGHTS+double-pumped MATMUL was the
        # bottleneck on HW)
        h_sb = const.tile([128, KH * BL], bf16)
        nc.gpsimd.memset(h_sb[:], 0.0)

        warm_ps = n_psum.tile([128, 128], f32, name="warm", tag="np")
        nc.tensor.transpose(warm_ps[:], ident[:], ident[:])

        gx_tiles = {}

        def make_units(c):
            """Emit-thunks for precomputing gx chunk c (16 steps)."""
            gx_t = gx_pool.tile([128, TC, M3H, BL], bf16, name="gx", tag="gx")
            gx_tiles[c] = gx_t
            xins = []
            xt_sb = xt_pool.tile([128, KD, TC * BL], bf16, name="xt", tag="xt")
            xt_ps = {}
            units = []

            def load(j):
                t = xin_pool.tile([128, D], f32, name="xin", tag="xin")
                xins.append(t)
                nc.sync.dma_start(
                    t[:],
                    x_view[c, 4 * j : 4 * (j + 1)],
                )

            def tr(j):
                # transpose both d-chunks of xin row-block j
                for kd in range(KD):
                    if j == 0:
                        xt_ps[kd] = xt_psum.tile([128, TC * BL], f32, name="xtp", tag="xtp")
                    nc.tensor.transpose(
                        xt_ps[kd][:, 128 * j : 128 * (j + 1)],
                        xins[j][:, 128 * kd : 128 * (kd + 1)],
                        ident[:],
                    )

            def evict_xt():
                for kd in range(KD):
                    nc.scalar.copy(xt_sb[:, kd, :], xt_ps[kd][:])

            def mm(m):
                ps = pre_psum.tile([128, TC * BL], f32, name="prep", tag="prep")
                for kd in range(KD):
                    nc.tensor.matmul(
                        ps[:],
                        w_sb[:, kd, m * 128 : (m + 1) * 128],
                        xt_sb[:, kd, :],
                        start=(kd == 0),
                        stop=(kd == KD - 1),
                    )
                # b folded into the eviction; alternate engines to balance
                # ACT/DVE load.
                if m % 2 == 0:
                    nc.scalar.activation(
                        gx_t[:, :, m, :],
                        ps[:].rearrange("p (t b) -> p t b", t=TC),
                        AF.Identity,
                        bias=b_sb[:, m : m + 1],
                    )
                else:
                    nc.vector.tensor_scalar(
                        gx_t[:, :, m, :],
                        ps[:].rearrange("p (t b) -> p t b", t=TC),
                        b_sb[:, m : m + 1],
                        None,
                        mybir.AluOpType.add,
                    )

            for j in range(4):
                units.append(lambda j=j: load(j))
            for j in range(4):
                units.append(lambda j=j: tr(j))
            units.append(evict_xt)
            for m in range(M3H):
                units.append(lambda m=m: mm(m))
            return units

        def emit_step(c, j):
            gx_t = gx_tiles[c]
            # PSUM pre-init on the PE itself: identity-stationary matmuls
            # stream the step's gx slices into the accumulators (start=True),
            # then the recurrent matmuls (start=False) accumulate on top.
            # Each gate group gets its OWN PSUM tile so reads unblock as soon
            # as that group's last matmul lands (reads are gated on the whole
            # accumulation group finishing, not the sub-region written).
            # Back-to-back identb matmuls share one LDWEIGHTS.
            r45_ps = r45_psum.tile([128, 2 * BL], f32, name="r45", tag="rp45")
            r67_ps = r67_psum.tile([128, 2 * BL], f32, name="r67", tag="rp67")
            z_ps = z_psum.tile([128, 4 * BL], f32, name="zp", tag="zp")
            n_ps = n_psum.tile([128, 4 * BL], f32, name="npt", tag="np")
            for ps, lo, hi in [
                (r45_ps, 4, 6),
                (r67_ps, 6, 8),
                (z_ps, 0, 4),
                (n_ps, 8, 12),
            ]:
                nc.tensor.matmul(
                    ps[:],
                    identb[:],
                    gx_t[:, j, lo:hi, :],
                    start=True,
                    stop=False,
                    skip_group_check=True,
                )

            # r-gate matmuls first (m=4+kk computes the r chunk for H-rows
            # kk), then z; the r -> rh -> n chain overlaps the z matmuls.
            def gate_mm(ps, col, m, last):
                for k in range(KH):
                    nc.tensor.matmul(
                        ps[:, col * BL : (col + 1) * BL],
                        u_sb[:, k, m * 128 : (m + 1) * 128],
                        h_sb[:, k * BL : (k + 1) * BL],
                        start=False,
                        stop=(last and k == KH - 1),
                        skip_group_check=True,
                    )

            gate_mm(r45_ps, 0, 4, False)
            gate_mm(r45_ps, 1, 5, True)
            gate_mm(r67_ps, 0, 6, False)
            gate_mm(r67_ps, 1, 7, True)
            for m in range(4):
                gate_mm(z_ps, m, m, m == 3)

            # r/rh in two half-chunks: first half unblocks the n-gate k=0/1
            # layers early, without paying 4x per-op overhead.
            r_sb = sb_pool.tile([128, 4 * BL], bf16, name="r", tag="r")
            rh_sb = sb_pool.tile([128, 4 * BL], bf16, name="rh", tag="rh")
            for half, ps in [(0, r45_ps), (1, r67_ps)]:
                sl = slice(half * 2 * BL, (half + 1) * 2 * BL)
                nc.scalar.activation(r_sb[:, sl], ps[:], AF.Sigmoid)
                nc.vector.tensor_mul(rh_sb[:, sl], r_sb[:, sl], h_sb[:, sl])

            # n-gate matmuls k-outer: layer k consumes rh chunk k.
            for k in range(KH):
                for m in range(4):
                    nc.tensor.matmul(
                        n_ps[:, m * BL : (m + 1) * BL],
                        u_sb[:, k, 1024 + m * 128 : 1024 + (m + 1) * 128],
                        rh_sb[:, k * BL : (k + 1) * BL],
                        start=False,
                        stop=(k == KH - 1),
                        skip_group_check=True,
                    )

            # SIGz before TANH on the (in-order) ACT queue: its input closes
            # earlier, and z feeds u=z*h which overlaps TANH on DVE.
            z_sb = sb_pool.tile([128, 4 * BL], bf16, name="z", tag="z")
            nc.scalar.activation(z_sb[:], z_ps[:], AF.Sigmoid)
            n_sb = sb_pool.tile([128, 4 * BL], bf16, name="n", tag="n")
            nc.scalar.activation(n_sb[:], n_ps[:], AF.Tanh)

            # h = z*h + (1-z)*n, computed as u=z*h (overlaps TANH),
            # v = (z-1)*n (one fused scalar_tensor_tensor), h = u - v.
            u_sb2 = sb_pool.tile([128, 4 * BL], bf16, name="u", tag="u")
            nc.vector.tensor_mul(u_sb2[:], z_sb[:], h_sb[:])
            v_sb = sb_pool.tile([128, 4 * BL], bf16, name="v", tag="v")
            nc.vector.scalar_tensor_tensor(
                v_sb[:],
                z_sb[:],
                1.0,
                n_sb[:],
                mybir.AluOpType.subtract,
                mybir.AluOpType.mult,
            )
            nc.vector.tensor_sub(h_sb[:], u_sb2[:], v_sb[:])

        # ---- main emission ----
        # Chunk 0's precompute up front; chunk c+1's precompute interleaved
        # between chunk c's recurrence steps so it fills TensorE gaps.
        for u in make_units(0):
            u()
        for c in range(nchunk):
            pend = make_units(c + 1) if c + 1 < nchunk else []
            done = 0
            for j in range(TC):
                emit_step(c, j)
                want = (len(pend) * (j + 1) + TC - 1) // TC
                while done < min(want, len(pend)):
                    pend[done]()
                    done += 1
            while done < len(pend):
                pend[done]()
                done += 1

        # final dense head: y = h @ Wd + bd
        out_ps = n_psum.tile([BL, 1], f32, name="outp", tag="np")
        for k in range(KH):
            nc.tensor.matmul(
                out_ps[:],
                h_sb[:, k * BL : (k + 1) * BL],
                wd_sb[:, k : k + 1],
                start=(k == 0),
                stop=False,
            )
        nc.tensor.matmul(
            out_ps[:], ones_sb[0:1, :], bd_sb[0:1, :], start=False, stop=True
        )
        y_sb = sb_pool.tile([BL, 1], f32, name="y", tag="y")
        nc.vector.tensor_copy(y_sb[:], out_ps[:])
        nc.sync.dma_start(y_d[:], y_sb[:])

    nc.compile()
    return nc


def kernel(x, W, U, b, Wd, bd):
    from concourse.bass_utils import run_bass_kernel_spmd

    t_run = int(os.environ.get("GRU_T_RUN", T))
    key = t_run
    if key not in _CACHE:
        _CACHE[key] = _build(t_run)
    nc = _CACHE[key]

    x = np.ascontiguousarray(np.asarray(x, dtype=np.float32))
    W = np.ascontiguousarray(np.asarray(W, dtype=np.float32))
    U = np.ascontiguousarray(np.asarray(U, dtype=np.float32))
    b = np.ascontiguousarray(np.asarray(b, dtype=np.float32))
    Wd = np.ascontiguousarray(np.asarray(Wd, dtype=np.float32))
    bd = np.ascontiguousarray(np.asarray(bd, dtype=np.float32))

    in_maps = [
        {
            "x": np.ascontiguousarray(x[i * BL : (i + 1) * BL]),
            "W": W,
            "U": U,
            "b": b,
            "Wd": Wd,
            "bd": bd,
        }
        for i in range(NCORES)
    ]
    res = run_bass_kernel_spmd(
        nc,
        in_maps,
        core_ids=list(range(NCORES)),
        trace=os.environ.get("GRU_TRACE", "0") == "1",
    )
    out = np.concatenate([r["y"] for r in res.results], axis=0)
    if res.exec_time_ns is not None:
        print(f"HW exec time: {res.exec_time_ns} ns")
    return out



# revision 19
# speedup vs baseline: 7.2830x; 1.2425x over previous
"""Trainium2 Bass kernel for CustomGRUModel.

Reference computation (per batch row):
    gx = x @ W                       # [T, 3H] input projections (precomputed)
    per step t:
        gh_zr = h @ U[:, :2H]
        z = sigmoid(gxz + ghz + bz)
        r = sigmoid(gxr + ghr + br)
        n = tanh(gxn + (r*h) @ U[:, 2H:] + bn)
        h = z*h + (1-z)*n
    y = h_last @ Wd + bd

Sharding: data-parallel over batch, 32 rows per core on 8 cores. Weights
replicated. No collectives.

Per-core layout: everything transposed ("feature on partitions"):
  hT [H=512, B=32] stored as one SBUF tile [128, 4*32] (4 H-chunks packed in
  the free dim). Recurrent matmuls keep U as the stationary operand
  (lhsT = U k-tile slice [128, 128], fp32 exact) streaming hT chunks (N=32):
  output lands transposed [3H-chunk, B] in PSUM, which makes the gate
  elementwise work run on full 128 partitions.

The gx precompute runs chunked (16 steps at a time) in float32r (1 cyc/row at
N=512), interleaved between recurrence steps so it fills TensorE gaps. x is
transposed on-chip with PE transposes. The bias b is folded into the
PSUM->SBUF eviction of gx (ACT activation bias).
"""

import os

import numpy as np

B, T, D, H = 256, 512, 256, 512
NCORES = 8
BL = B // NCORES  # 32 batch rows per core
TC = 16  # timestep chunk for the gx precompute
KH = H // 128  # 4 k-tiles over H
KD = D // 128  # 2 k-tiles over D
M3H = 3 * H // 128  # 12 m-tiles over 3H

_CACHE = {}


def _build(t_run):
    from contextlib import ExitStack

    import concourse.bacc as bacc
    import concourse.bass as bass
    import concourse.tile as tile
    from concourse import masks, mybir

    dt = mybir.dt
    f32 = dt.float32
    f32r = dt.float32r
    bf16 = dt.bfloat16
    AF = mybir.ActivationFunctionType

    nchunk = t_run // TC

    nc = bacc.Bacc(
        "TRN2", target_bir_lowering=False, debug=False, num_devices=NCORES
    )
    x_d = nc.dram_tensor("x", [BL, T, D], f32, kind="ExternalInput")
    w_d = nc.dram_tensor("W", [D, 3 * H], f32, kind="ExternalInput")
    u_d = nc.dram_tensor("U", [H, 3 * H], f32, kind="ExternalInput")
    b_d = nc.dram_tensor("b", [3 * H], f32, kind="ExternalInput")
    wd_d = nc.dram_tensor("Wd", [H, 1], f32, kind="ExternalInput")
    bd_d = nc.dram_tensor("bd", [1], f32, kind="ExternalInput")
    y_d = nc.dram_tensor("y", [BL, 1], f32, kind="ExternalOutput")

    # chunked view of x: [chunk, tc, b, d]
    x_view = x_d.rearrange("b (c t) d -> c t b d", t=TC)

    with tile.TileContext(nc) as tc, ExitStack() as ctx:
        const = ctx.enter_context(tc.tile_pool(name="const", bufs=1))
        gx_pool = ctx.enter_context(tc.tile_pool(name="gx", bufs=2))
        xin_pool = ctx.enter_context(tc.tile_pool(name="xin", bufs=8))
        xt_pool = ctx.enter_context(tc.tile_pool(name="xt", bufs=2))
        sb_pool = ctx.enter_context(tc.tile_pool(name="sb", bufs=3))
        r45_psum = ctx.enter_context(
            tc.tile_pool(name="rp45", bufs=1, space=bass.MemorySpace.PSUM)
        )
        r67_psum = ctx.enter_context(
            tc.tile_pool(name="rp67", bufs=1, space=bass.MemorySpace.PSUM)
        )
        z_psum = ctx.enter_context(
            tc.tile_pool(name="zp", bufs=1, space=bass.MemorySpace.PSUM)
        )
        n_psum = ctx.enter_context(
            tc.tile_pool(name="np", bufs=1, space=bass.MemorySpace.PSUM)
        )
        pre_psum = ctx.enter_context(
            tc.tile_pool(name="prep", bufs=2, space=bass.MemorySpace.PSUM)
        )
        xt_psum = ctx.enter_context(
            tc.tile_pool(name="xtp", bufs=2, space=bass.MemorySpace.PSUM)
        )

        # ---- constants ----
        w_stage = const.tile([128, KD, 3 * H], f32)
        for k in range(KD):
            nc.sync.dma_start(w_stage[:, k, :], w_d[k * 128 : (k + 1) * 128, :])
        w_sb = const.tile([128, KD, 3 * H], bf16)
        for k in range(KD):
            nc.scalar.copy(w_sb[:, k, :], w_stage[:, k, :])
        u_stage = const.tile([128, KH, 3 * H], f32)
        for k in range(KH):
            nc.sync.dma_start(u_stage[:, k, :], u_d[k * 128 : (k + 1) * 128, :])
        u_sb = const.tile([128, KH, 3 * H], bf16)
        for k in range(KH):
            nc.scalar.copy(u_sb[:, k, :], u_stage[:, k, :])
        b_sb = const.tile([128, M3H], f32)
        nc.sync.dma_start(b_sb[:], b_d.rearrange("(m p) -> p m", p=128))
        wd_stage = const.tile([128, KH], f32)
        nc.sync.dma_start(wd_stage[:], wd_d.rearrange("(k p) o -> p (k o)", p=128))
        wd_sb = const.tile([128, KH], bf16)
        nc.scalar.copy(wd_sb[:], wd_stage[:])
        bd_stage = const.tile([1, 1], f32)
        nc.sync.dma_start(bd_stage[0:1, :], bd_d.rearrange("(o u) -> o u", u=1))
        bd_sb = const.tile([1, 1], bf16)
        nc.scalar.copy(bd_sb[0:1, :], bd_stage[0:1, :])
        ident = const.tile([128, 128], f32)
        masks.make_identity(nc, ident[:])
        identb = const.tile([128, 128], bf16)
        nc.scalar.copy(identb[:], ident[:])
        ones_sb = const.tile([1, BL], bf16)
        nc.gpsimd.memset(ones_sb[0:1, :], 1.0)

        # persistent hidden state hT: [128, (k, b)] = [128, 4*32], bf16
        # (matmul moving operand; fp32 LDWEIGHTS+double-pumped MATMUL was the
        # bottleneck on HW)
        h_sb = const.tile([128, KH * BL], bf16)
        nc.gpsimd.memset(h_sb[:], 0.0)

        warm_ps = n_psum.tile([128, 128], f32, name="warm", tag="np")
        nc.tensor.transpose(warm_ps[:], ident[:], ident[:])

        gx_tiles = {}

        def make_units(c):
            """Emit-thunks for precomputing gx chunk c (16 steps)."""
            gx_t = gx_pool.tile([128, TC, M3H, BL], bf16, name="gx", tag="gx")
            gx_tiles[c] = gx_t
            xins = []
            xt_sb = xt_pool.tile([128, KD, TC * BL], bf16, name="xt", tag="xt")
            xt_ps = {}
            units = []

            def load(j):
                t = xin_pool.tile([128, D], f32, name="xin", tag="xin")
                xins.append(t)
                nc.sync.dma_start(
                    t[:],
                    x_view[c, 4 * j : 4 * (j + 1)],
                )

            def tr(j):
                # transpose both d-chunks of xin row-block j
                for kd in range(KD):
                    if j == 0:
                        xt_ps[kd] = xt_psum.tile([128, TC * BL], f32, name="xtp", tag="xtp")
                    nc.tensor.transpose(
                        xt_ps[kd][:, 128 * j : 128 * (j + 1)],
                        xins[j][:, 128 * kd : 128 * (kd + 1)],
                        ident[:],
                    )

            def evict_xt(kd, half):
                sl = slice(half * TC * BL // 2, (half + 1) * TC * BL // 2)
                if half == 0:
                    nc.scalar.copy(xt_sb[:, kd, sl], xt_ps[kd][:, sl])
                else:
                    nc.vector.tensor_copy(xt_sb[:, kd, sl], xt_ps[kd][:, sl])

            prep_tiles = {}

            def mm(m):
                ps = pre_psum.tile([128, TC * BL], f32, name="prep", tag="prep")
                prep_tiles[m] = ps
                for kd in range(KD):
                    nc.tensor.matmul(
                        ps[:],
                        w_sb[:, kd, m * 128 : (m + 1) * 128],
                        xt_sb[:, kd, :],
                        start=(kd == 0),
                        stop=(kd == KD - 1),
                    )

            def evict(m):
                # b folded into the eviction; the two halves run in parallel
                # on ACT and DVE so neither in-order queue eats the full cost.
                ps = prep_tiles.pop(m)
                hc = TC // 2
                nc.scalar.activation(
                    gx_t[:, 0:hc, m, :],
                    ps[:, 0 : hc * BL].rearrange("p (t b) -> p t b", t=hc),
                    AF.Identity,
                    bias=b_sb[:, m : m + 1],
                )
                nc.vector.tensor_scalar(
                    gx_t[:, hc:TC, m, :],
                    ps[:, hc * BL : TC * BL].rearrange("p (t b) -> p t b", t=hc),
                    b_sb[:, m : m + 1],
                    None,
                    mybir.AluOpType.add,
                )

            for j in range(4):
                units.append(lambda j=j: load(j))
            for j in range(4):
                units.append(lambda j=j: tr(j))
            for kd in range(KD):
                for half in range(2):
                    units.append(lambda kd=kd, half=half: evict_xt(kd, half))
            for m in range(M3H):
                units.append(lambda m=m: mm(m))
                units.append(lambda m=m: evict(m))
            return units

        def emit_step(c, j):
            gx_t = gx_tiles[c]
            # PSUM pre-init on the PE itself: identity-stationary matmuls
            # stream the step's gx slices into the accumulators (start=True),
            # then the recurrent matmuls (start=False) accumulate on top.
            # Each gate group gets its OWN PSUM tile so reads unblock as soon
            # as that group's last matmul lands (reads are gated on the whole
            # accumulation group finishing, not the sub-region written).
            # Back-to-back identb matmuls share one LDWEIGHTS.
            r45_ps = r45_psum.tile([128, 2 * BL], f32, name="r45", tag="rp45")
            r67_ps = r67_psum.tile([128, 2 * BL], f32, name="r67", tag="rp67")
            z_ps = z_psum.tile([128, 4 * BL], f32, name="zp", tag="zp")
            n_ps = n_psum.tile([128, 4 * BL], f32, name="npt", tag="np")
            for ps, lo, hi in [
                (r45_ps, 4, 6),
                (r67_ps, 6, 8),
                (z_ps, 0, 4),
                (n_ps, 8, 12),
            ]:
                nc.tensor.matmul(
                    ps[:],
                    identb[:],
                    gx_t[:, j, lo:hi, :],
                    start=True,
                    stop=False,
                    skip_group_check=True,
                )

            # r-gate matmuls first (m=4+kk computes the r chunk for H-rows
            # kk), then z; the r -> rh -> n chain overlaps the z matmuls.
            def gate_mm(ps, col, m, last):
                for k in range(KH):
                    nc.tensor.matmul(
                        ps[:, col * BL : (col + 1) * BL],
                        u_sb[:, k, m * 128 : (m + 1) * 128],
                        h_sb[:, k * BL : (k + 1) * BL],
                        start=False,
                        stop=(last and k == KH - 1),
                        skip_group_check=True,
                    )

            gate_mm(r45_ps, 0, 4, False)
            gate_mm(r45_ps, 1, 5, True)
            gate_mm(r67_ps, 0, 6, False)
            gate_mm(r67_ps, 1, 7, True)
            for m in range(4):
                gate_mm(z_ps, m, m, m == 3)

            # r/rh in two half-chunks: first half unblocks the n-gate k=0/1
            # layers early, without paying 4x per-op overhead.
            r_sb = sb_pool.tile([128, 4 * BL], bf16, name="r", tag="r")
            rh_sb = sb_pool.tile([128, 4 * BL], bf16, name="rh", tag="rh")
            for half, ps in [(0, r45_ps), (1, r67_ps)]:
                sl = slice(half * 2 * BL, (half + 1) * 2 * BL)
                nc.scalar.activation(r_sb[:, sl], ps[:], AF.Sigmoid)
                nc.vector.tensor_mul(rh_sb[:, sl], r_sb[:, sl], h_sb[:, sl])

            # n-gate matmuls k-outer: layer k consumes rh chunk k.
            for k in range(KH):
                for m in range(4):
                    nc.tensor.matmul(
                        n_ps[:, m * BL : (m + 1) * BL],
                        u_sb[:, k, 1024 + m * 128 : 1024 + (m + 1) * 128],
                        rh_sb[:, k * BL : (k + 1) * BL],
                        start=False,
                        stop=(k == KH - 1),
                        skip_group_check=True,
                    )

            # SIGz before TANH on the (in-order) ACT queue: its input closes
            # earlier, and z feeds u=z*h which overlaps TANH on DVE.
            z_sb = sb_pool.tile([128, 4 * BL], bf16, name="z", tag="z")
            nc.scalar.activation(z_sb[:], z_ps[:], AF.Sigmoid)
            n_sb = sb_pool.tile([128, 4 * BL], bf16, name="n", tag="n")
            nc.scalar.activation(n_sb[:], n_ps[:], AF.Tanh)

            # h = z*h + (1-z)*n, computed as u=z*h (overlaps TANH),
            # v = (z-1)*n (one fused scalar_tensor_tensor), h = u - v.
            u_sb2 = sb_pool.tile([128, 4 * BL], bf16, name="u", tag="u")
            nc.vector.tensor_mul(u_sb2[:], z_sb[:], h_sb[:])
            v_sb = sb_pool.tile([128, 4 * BL], bf16, name="v", tag="v")
            nc.vector.scalar_tensor_tensor(
                v_sb[:],
                z_sb[:],
                1.0,
                n_sb[:],
                mybir.AluOpType.subtract,
                mybir.AluOpType.mult,
            )
            nc.vector.tensor_sub(h_sb[:], u_sb2[:], v_sb[:])

        # ---- main emission ----
        # Chunk 0's precompute up front; chunk c+1's precompute interleaved
        # between chunk c's recurrence steps so it fills TensorE gaps.
        for u in make_units(0):
            u()
        for c in range(nchunk):
            pend = make_units(c + 1) if c + 1 < nchunk else []
            done = 0
            for j in range(TC):
                # Emit precompute units BEFORE the step: their PE matmuls run
                # in the PE-idle tail of the previous step, and the evictions
                # land on ACT/DVE with inputs already available, ahead of the
                # step's chain ops in the in-order queues.
                want = (len(pend) * (j + 1) + TC - 1) // TC
                while done < min(want, len(pend)):
                    pend[done]()
                    done += 1
                emit_step(c, j)
            while done < len(pend):
                pend[done]()
                done += 1

        # final dense head: y = h @ Wd + bd
        out_ps = n_psum.tile([BL, 1], f32, name="outp", tag="np")
        for k in range(KH):
            nc.tensor.matmul(
                out_ps[:],
                h_sb[:, k * BL : (k + 1) * BL],
                wd_sb[:, k : k + 1],
                start=(k == 0),
                stop=False,
            )
        nc.tensor.matmul(
            out_ps[:], ones_sb[0:1, :], bd_sb[0:1, :], start=False, stop=True
        )
        y_sb = sb_pool.tile([BL, 1], f32, name="y", tag="y")
        nc.vector.tensor_copy(y_sb[:], out_ps[:])
        nc.sync.dma_start(y_d[:], y_sb[:])

    nc.compile()
    return nc


def kernel(x, W, U, b, Wd, bd):
    from concourse.bass_utils import run_bass_kernel_spmd

    t_run = int(os.environ.get("GRU_T_RUN", T))
    key = t_run
    if key not in _CACHE:
        _CACHE[key] = _build(t_run)
    nc = _CACHE[key]

    x = np.ascontiguousarray(np.asarray(x, dtype=np.float32))
    W = np.ascontiguousarray(np.asarray(W, dtype=np.float32))
    U = np.ascontiguousarray(np.asarray(U, dtype=np.float32))
    b = np.ascontiguousarray(np.asarray(b, dtype=np.float32))
    Wd = np.ascontiguousarray(np.asarray(Wd, dtype=np.float32))
    bd = np.ascontiguousarray(np.asarray(bd, dtype=np.float32))

    in_maps = [
        {
            "x": np.ascontiguousarray(x[i * BL : (i + 1) * BL]),
            "W": W,
            "U": U,
            "b": b,
            "Wd": Wd,
            "bd": bd,
        }
        for i in range(NCORES)
    ]
    res = run_bass_kernel_spmd(
        nc,
        in_maps,
        core_ids=list(range(NCORES)),
        trace=os.environ.get("GRU_TRACE", "0") == "1",
    )
    out = np.concatenate([r["y"] for r in res.results], axis=0)
    if res.exec_time_ns is not None:
        print(f"HW exec time: {res.exec_time_ns} ns")
    return out

